# revision 24
# baseline (speedup 1.0000x reference)
"""Trainium2 Bass kernel for nn_BaseIODEModel (GNN message-passing ODE field).

Data-parallel over trajectories: z [81920, 4] is split across 8 NeuronCores
along dim 0 in multiples of B=10 (1024 trajectories / 10240 rows per core);
the small MLP weights are replicated. Edge gather/softplus/sum is local per
trajectory, so there is no cross-device communication.

Per-core program (feature-major on chip; bf16/fp16 datapath, fp32 PSUM):
  zT = transpose(z)                                 [4, cols]   (PE transpose)
  self-dynamics MLP:   softplus = ln(1 + exp(.)) via the ACT engine's
                       exp/ln table set (walrus has no native softplus set)
  interaction net: layer-0 factorizes over edges:
       pre(r,s) = a_r + b_s + ib0 with a = [iW0_p; iW0_vr].T z,
                                       b = [-iW0_p; iW0_vs].T z
       exp(pre) = exp(a + ib0/2) * exp(b + ib0/2)  -> exp on node cols, not
                                                      edge cols; grid combine
                                                      is a DVE fp16 multiply
       h0e = ln(1 + ea_r * eb_s)   (grid (d, r), s = (r+d) mod 10, d=1..9)
       u_d = exp(iW1.T h0e + ib1)              [2nd edge-sized ACT pass]
       sum_d softplus(iW1.T h0e + ib1) = ln( prod_d (1 + u_d) )
           -> the product runs on the DVE as a fused (u+1)*acc chain, so the
              second softplus costs one edge-sized exp + one node-sized ln
              instead of edge-sized exp + edge-sized ln.
       dz_int = iW2.T ln(prod)                 [node-sized matmul]
  out = fW2.T h1s + dz_int + (fb2 + 9*ib2), PE-transposed back to row-major.
"""

import numpy as np

import concourse.bass as bass
import concourse.hw_specs as _hw_specs
import concourse.mybir as _mybir_for_tables
from concourse import bacc


def _patch_activation_tables():
    """Make Exp and Ln resolve to the combined natural_log_exp_and_others
    ACT table set. Bacc's insert_act_table_loads picks the first set that
    contains each function, which puts Exp and Ln in two different sets and
    inserts a ~1.3us ACT_TABLE_LOAD at every exp<->ln alternation. Filtering
    the other sets' exp/ln entries keeps set ids stable (index into
    act_info.json) while forcing the shared set."""
    if getattr(_hw_specs, "_nle_patched", False):
        return
    orig = _hw_specs.get_activation_tables
    comb = "natural_log_exp_and_others"
    EXP = _mybir_for_tables.ActivationFunctionType.Exp
    LN = _mybir_for_tables.ActivationFunctionType.Ln

    def patched(module_arch):
        tables = orig(module_arch)
        if comb in tables and EXP in tables[comb] and LN in tables[comb]:
            for name, funcs in tables.items():
                if name != comb:
                    funcs.discard(EXP)
                    funcs.discard(LN)
        return tables

    _hw_specs.get_activation_tables = patched
    _hw_specs._nle_patched = True
    import concourse.bacc as _bacc_mod
    if getattr(_bacc_mod, "get_activation_tables", None) is orig:
        _bacc_mod.get_activation_tables = patched


_patch_activation_tables()
import concourse.mybir as mybir
import concourse.tile as tile
from concourse.alu_op_type import AluOpType
from concourse.bass_utils import run_bass_kernel_spmd
from concourse.masks import make_identity

F32 = mybir.dt.float32
F16 = mybir.dt.float16
BF16 = mybir.dt.bfloat16
AF = mybir.ActivationFunctionType

B = 10          # objects per trajectory
NDIM = 2
NF = 2 * NDIM   # 4 features per node
H = 128         # hidden width (both MLPs)
ND = B - 1      # senders per receiver

N_CORES = 8
N_TRAJ = 8192           # total trajectories
N_LOC = N_TRAJ // N_CORES  # 1024 trajectories per core
ROWS = N_LOC * B        # 10240 node rows per core
GT = 128                # trajectories per group
NGROUP = N_LOC // GT    # 8 groups
GCOLS = GT * B          # 1280 node cols per group
TT = 32                 # trajectories per edge block
NBLK = GT // TT         # 4 edge blocks per group
BCOLS = TT * B          # 320 node cols per block
GRID = TT * ND * B      # 2880 grid cols per block

WEIGHT_NAMES = [
    "fW0", "fb0", "fW1", "fb1", "fW2", "fb2",
    "iW0", "ib0", "iW1", "ib1", "iW2", "ib2",
]


def build(ngroup=NGROUP):
    nc = bacc.Bacc()
    rows = ngroup * GCOLS

    z = nc.declare_dram_parameter("z", [rows, NF], F32, isOutput=False)
    w = {}
    for name, shp in [
        ("fW0", [NF, H]), ("fb0", [H]), ("fW1", [H, H]), ("fb1", [H]),
        ("fW2", [H, NF]), ("fb2", [NF]),
        ("iW0", [3 * NDIM, H]), ("ib0", [H]), ("iW1", [H, H]), ("ib1", [H]),
        ("iW2", [H, NF]), ("ib2", [NF]),
        ("Wb", [NF, H]), ("bias2", [NF]), ("ib0h", [H]),
    ]:
        w[name] = nc.declare_dram_parameter(name, shp, F32, isOutput=False)
    out = nc.declare_dram_parameter("out", [rows, NF], F32, isOutput=True)

    # DRAM views: rows=(g,p,c): partition p = trajectory, c = node.
    # Per-partition runs are 10*4 contiguous f32 (160B DMA bursts).
    z_v = z.rearrange("(g p c) f -> g p (c f)", g=ngroup, p=128, c=B)
    out_v = out.rearrange("(g p c) f -> g p (c f)", g=ngroup, p=128, c=B)

    with tile.TileContext(nc) as tc:
        with (
            tc.tile_pool(name="const", bufs=1) as const,
            tc.tile_pool(name="zio", bufs=2) as zio,
            tc.tile_pool(name="nodes", bufs=3) as nodes,
            tc.tile_pool(name="grids", bufs=2) as grids,
            tc.tile_pool(name="outs", bufs=2) as outs,
            tc.tile_pool(name="misc_ps", bufs=1, space="PSUM") as misc_ps,
            tc.tile_pool(name="ab_ps", bufs=1, space="PSUM") as ab_ps,
            tc.tile_pool(name="edge_ps", bufs=2, space="PSUM") as edge_ps,
            tc.tile_pool(name="dz_ps", bufs=1, space="PSUM") as dz_ps,
        ):
            # ---- constants / weights ----
            ident128 = const.tile([128, 128], F32)
            make_identity(nc, ident128)
            ident4 = const.tile([NF, NF], F32)
            make_identity(nc, ident4)
            _zT0 = [None]

            def z_load(g):
                # ---- load z (contiguous) and transpose to feature-major ----
                z_sb = zio.tile([128, B, NF], F32)  # [traj, node, feat]
                nc.sync.dma_start(out=z_sb[:].rearrange("p c f -> p (c f)"),
                                  in_=z_v[g])

                # zT cols ordered (node r, traj t): col = r*128 + t
                zT_sb = zio.tile([NF, GCOLS], BF16)
                for h in range(3):  # col chunks of 512,512,256
                    c0 = h * 512
                    c1 = min(GCOLS, c0 + 512)
                    zt_ps = misc_ps.tile([128, 512], F32, tag="misc")
                    for r in range(c0 // 128, c1 // 128):
                        nc.tensor.transpose(
                            zt_ps[0:NF, r * 128 - c0:(r + 1) * 128 - c0],
                            z_sb[:, r, :],
                            ident128[:],
                        )
                    nc.vector.tensor_copy(zT_sb[:, c0:c1], zt_ps[0:NF, 0:c1 - c0])
                return zT_sb

            def node_phase(g, zT_sb=None):
                if zT_sb is None:
                    zT_sb = z_load(g)
                # ---- node terms: ea = exp(a+ib0/2), eb = exp(b+ib0/2) ----
                # packed in one tile so each chunk is a single ACT instr
                eab = nodes.tile([H, 2, B, GT], F16)      # [h, a|b, r, t]
                ea_sb = eab[:, 0]
                eb_f = eab[:].rearrange("p s r t -> p (s r t)")
                eab_v = eab[:].rearrange("p s r t -> p s (r t)")
                for h in range(3):
                    c0 = h * 512
                    c1 = min(GCOLS, c0 + 512)
                    wd = c1 - c0
                    ab2_ps = ab_ps.tile([128, 2, 512], F32, tag="ab2")
                    nc.tensor.matmul(
                        ab2_ps[:, 0, 0:wd], Wa_sb[:], zT_sb[:, c0:c1])
                    nc.tensor.matmul(
                        ab2_ps[:, 1, 0:wd], Wb_sb[:], zT_sb[:, c0:c1])
                    nc.scalar.activation(
                        out=eab_v[:, :, c0:c1],
                        in_=ab2_ps[:, :, 0:wd], func=AF.Exp,
                        bias=ib0h_c[:], scale=1.0)
                # duplicate eb for cyclic sender indexing:
                # eb_ext[:, j] = eb[:, j % B]
                eb_ext = nodes.tile([H, 2 * B, GT], F16)
                nc.vector.tensor_copy(
                    eb_ext[:].rearrange("p r t -> p (r t)")[:, 0:GCOLS],
                    eb_f[:, GCOLS:2 * GCOLS])
                nc.vector.tensor_copy(
                    eb_ext[:].rearrange("p r t -> p (r t)")[:, GCOLS:2 * GCOLS],
                    eb_f[:, GCOLS:2 * GCOLS])

                # ---- self MLP (feature-major) ----
                h1s_sb = nodes.tile([H, GCOLS], BF16)
                for h in range(3):
                    c0 = h * 512
                    c1 = min(GCOLS, c0 + 512)
                    wd = c1 - c0
                    s0_full = ab_ps.tile([128, 2, 512], F32, tag="ab2")
                    s0_ps = s0_full[:, 0]
                    nc.tensor.matmul(s0_ps[:, 0:wd], fW0_sb[:], zT_sb[:, c0:c1])
                    t0s = zio.tile([H, 512], F16, tag="t0s")
                    nc.scalar.activation(out=t0s[:, 0:wd], in_=s0_ps[:, 0:wd],
                                         func=AF.Exp, bias=fb0_c[:], scale=1.0)
                    h0s = zio.tile([H, 512], BF16, tag="h0s")
                    nc.scalar.activation(out=h0s[:, 0:wd], in_=t0s[:, 0:wd],
                                         func=AF.Ln, bias=1.0, scale=1.0)
                    s1_full = ab_ps.tile([128, 2, 512], F32, tag="ab2")
                    s1_ps = s1_full[:, 0]
                    nc.tensor.matmul(s1_ps[:, 0:wd], fW1_sb[:], h0s[:, 0:wd])
                    t1s = zio.tile([H, 512], F16, tag="t1s")
                    nc.scalar.activation(out=t1s[:, 0:wd], in_=s1_ps[:, 0:wd],
                                         func=AF.Exp, bias=fb1_c[:], scale=1.0)
                    nc.scalar.activation(out=h1s_sb[:, c0:c1], in_=t1s[:, 0:wd],
                                         func=AF.Ln, bias=1.0, scale=1.0)
                return ea_sb, eb_ext, h1s_sb

            _pending = [None]

            def grid_mul_group(ea_sb, eb_ext):
                # DVE fp16 grid combine for a whole group: 9 wide multiplies
                # (one per sender offset) instead of 36 per-block ones.
                t0g = grids.tile([H, ND, B, GT], F16)  # (d, r, t)
                for d in range(1, B):
                    nc.vector.tensor_mul(
                        t0g[:, d - 1, :, :],
                        ea_sb[:],
                        eb_ext[:, d:d + B, :],
                    )
                return t0g

            def grid_ln(k, t0g):
                # ACT ln(1+x) for block k, reading a strided t-slice of the
                # group grid tile; emitted one block ahead so ACT has this
                # while PE runs L1 matmuls.
                tsl = slice(k * TT, (k + 1) * TT)
                g0 = grids.tile([H, GRID], BF16)
                nc.scalar.activation(
                    out=g0[:].rearrange("p (d r t) -> p d r t", d=ND, r=B),
                    in_=t0g[:, :, :, tsl], func=AF.Ln, bias=1.0, scale=1.0)
                return g0

            def edge_phase(g, ea_sb, eb_ext, h1s_sb, prefetch_g=None,
                           pre=None):
                # u[d-1, r, t] = exp(iW1.T h0e + ib1) for sender offset d
                u_sb = grids.tile([H, ND, B, GT], BF16)
                nxt = None
                nxt_pre = None

                if pre is None:
                    t0g = grid_mul_group(ea_sb, eb_ext)
                    g0 = grid_ln(0, t0g)
                else:
                    t0g, g0 = pre
                for k in range(NBLK):
                    if k == 1:
                        # previous group's DVE product chain is emitted here
                        # (after this group's first grid blocks are queued);
                        # its ACT lnP + L2 + store are deferred two more
                        # blocks (end of this edge_phase) so the in-order ACT
                        # queue never stalls waiting on the chain.
                        if _pending[0] is not None:
                            _pending[0]["acc"] = _chain_part(
                                _pending[0]["u_sb"])
                        if prefetch_g is not None:
                            nxt = node_phase(prefetch_g)
                    tsl = slice(k * TT, (k + 1) * TT)
                    if k + 1 < NBLK:
                        g0_next = grid_ln(k + 1, t0g)
                    else:
                        g0_next = None
                        if nxt is not None:
                            # next group's grid multiplies + first ln block,
                            # queued ahead of this group's chain/finish
                            t0g_n = grid_mul_group(nxt[0], nxt[1])
                            nxt_pre = (t0g_n, grid_ln(0, t0g_n))
                    for third in range(3):
                        e_ps = edge_ps.tile([128, 960], F32)
                        base = third * 960
                        for q0, q1 in [(0, 512), (512, 960)]:
                            nc.tensor.matmul(
                                e_ps[:, q0:q1],
                                iW1_sb[:],
                                g0[:, base + q0:base + q1])
                        nc.scalar.activation(
                            out=u_sb[:, 3 * third:3 * third + 3, :, tsl],
                            in_=e_ps[:].rearrange("p (d c) -> p d c", d=3),
                            func=AF.Exp, bias=ib1_c[:], scale=1.0)
                    g0 = g0_next

                if _pending[0] is not None:
                    _tail_part(_pending[0]["g"], _pending[0]["acc"],
                               _pending[0]["h1s"])
                _pending[0] = {"g": g, "u_sb": u_sb, "h1s": h1s_sb,
                               "acc": None}
                return nxt, nxt_pre

            def _chain_part(u_sb):
                # ---- product chain over senders: acc = prod_d (1 + u_d) ----
                # Pre-scaled by 2^-36 to keep the product inside the Ln
                # table's accurate range [3.5e-20, 2.6e19] (the raw product
                # reaches ~e^50 in the tails); 36*ln2 is added back after.
                acc = grids.tile([H, B, GT], BF16, tag="accA")
                nc.vector.tensor_scalar(
                    out=acc[:], in0=u_sb[:, 0], scalar1=1.0, scalar2=2.0 ** -36,
                    op0=AluOpType.add, op1=AluOpType.mult)
                for d in range(1, ND):
                    nxt_acc = grids.tile([H, B, GT], BF16,
                                         tag="accB" if d % 2 else "accA")
                    nc.vector.scalar_tensor_tensor(
                        out=nxt_acc[:], in0=u_sb[:, d], scalar=1.0,
                        in1=acc[:], op0=AluOpType.add, op1=AluOpType.mult)
                    acc = nxt_acc
                return acc

            def _tail_part(g, acc, h1s_sb):
                # sum_d h1e[d] = ln(acc) + 36*ln2; the constant is folded
                # into bias2 host-side via iW2.T (36*ln2*ones) so Ln can
                # write f16 directly (range ~[-25, 8], quant ~0.012).
                sum_h1 = grids.tile([H, B, GT], F16, tag="sumh1")
                nc.scalar.activation(
                    out=sum_h1[:], in_=acc[:], func=AF.Ln, scale=1.0)
                sum_f = sum_h1[:].rearrange("p r t -> p (r t)")

                # ---- L2: dz = fW2.T h1s + iW2.T sum_h1 + bias2 ----
                out_sb = outs.tile([NF, B, GT], F32)  # (r, t)
                out_f = out_sb[:].rearrange("p r t -> p (r t)")
                for h in range(3):
                    c0 = h * 512
                    c1 = min(GCOLS, c0 + 512)
                    wd = c1 - c0
                    dzp = dz_ps.tile([NF, 512], F32)
                    nc.tensor.matmul(dzp[:, 0:wd], fW2_sb[:],
                                     h1s_sb[:, c0:c1], start=True, stop=False)
                    nc.tensor.matmul(dzp[:, 0:wd], iW2_sb[:],
                                     sum_f[:, c0:c1], start=False, stop=True)
                    nc.vector.tensor_scalar_add(
                        out_f[:, c0:c1], dzp[:, 0:wd], bias2[:])

                # ---- transpose back and store (contiguous) ----
                ot_ps = misc_ps.tile([128, 512], F32, tag="misc")
                for r in range(B):
                    nc.tensor.transpose(
                        ot_ps[:, r * NF:(r + 1) * NF],
                        out_f[:, r * 128:(r + 1) * 128],
                        ident4[:],
                    )
                outT_sb = outs.tile([128, B, NF], F32)
                nc.vector.tensor_copy(outT_sb[:], ot_ps[:, 0:B * NF])
                nc.sync.dma_start(out=out_v[g],
                                  in_=outT_sb[:].rearrange("p c f -> p (c f)"))

            _zT0[0] = z_load(0)

            def weight_tile(p, fdim, name, src_ap):
                # DMA to fp32 staging, round to bf16 on DVE for the PE.
                stage = const.tile([p, fdim], F32, tag=f"wstage_{name}")
                nc.sync.dma_start(out=stage[:], in_=src_ap)
                t = const.tile([p, fdim], BF16, tag=f"w_{name}")
                nc.vector.tensor_copy(t[:], stage[:])
                return t

            fW0_sb = weight_tile(NF, H, "fW0", w["fW0"][:])
            fW1_sb = weight_tile(H, H, "fW1", w["fW1"][:])
            fW2_sb = weight_tile(H, NF, "fW2", w["fW2"][:])
            iW1_sb = weight_tile(H, H, "iW1", w["iW1"][:])
            iW2_sb = weight_tile(H, NF, "iW2", w["iW2"][:])

            # Wa = iW0[0:4]  (pos-part rows 0:2, v_recv rows 2:4)
            Wa_sb = weight_tile(NF, H, "Wa", w["iW0"][0:NF, :])
            # Wb = [-iW0[0:2]; iW0[4:6]] is prepared host-side (param "Wb")
            Wb_sb = weight_tile(NF, H, "Wb", w["Wb"][:])

            # bias columns [P,1]
            def bias_col(p, name):
                t = const.tile([p, 1], F32, tag=f"bias_{name}")
                nc.sync.dma_start(out=t[:], in_=w[name].rearrange("(a b) -> a b", b=1))
                return t

            fb0_c = bias_col(H, "fb0")
            fb1_c = bias_col(H, "fb1")
            ib0h_c = bias_col(H, "ib0h")   # ib0/2, host-side
            ib1_c = bias_col(H, "ib1")
            # bias2 = fb2 + 9*ib2 is prepared host-side (param "bias2")
            bias2 = bias_col(NF, "bias2")

            # software-pipelined: group g+1's node phase is emitted after
            # group g's first edge block, so its PE/DVE prefetch work runs
            # while ACT chews on g's grid, without blocking g's ACT queue.
            tiles = node_phase(0, _zT0[0])
            pre = None
            for g in range(ngroup):
                pf = g + 1 if g + 1 < ngroup else None
                tiles, pre = edge_phase(g, *tiles, prefetch_g=pf, pre=pre)
            p = _pending[0]
            p["acc"] = _chain_part(p["u_sb"])
            _tail_part(p["g"], p["acc"], p["h1s"])

    nc.finalize()
    return nc


_NC_CACHE = {}


def _get_nc():
    if "nc" not in _NC_CACHE:
        _NC_CACHE["nc"] = build()
    return _NC_CACHE["nc"]


def run(inputs, trace=False, **kwargs):
    """Shard, run on 8 cores, gather. Returns (out, BassKernelResults)."""
    nc = _get_nc()
    z = np.ascontiguousarray(np.asarray(inputs["z"], dtype=np.float32))
    assert z.shape == (N_TRAJ * B, NF), z.shape
    weights = {k: np.ascontiguousarray(np.asarray(inputs[k], dtype=np.float32))
               for k in WEIGHT_NAMES}
    iW0 = weights["iW0"]
    weights["Wb"] = np.ascontiguousarray(
        np.concatenate([-iW0[0:NDIM], iW0[2 * NDIM:3 * NDIM]], axis=0))
    weights["bias2"] = np.ascontiguousarray(
        weights["fb2"] + (B - 1) * weights["ib2"]
        + np.float32(36 * np.log(2.0)) * weights["iW2"].sum(axis=0))
    weights["ib0h"] = np.ascontiguousarray(0.5 * weights["ib0"])
    in_maps = []
    for c in range(N_CORES):
        m = dict(weights)
        m["z"] = z[c * ROWS:(c + 1) * ROWS]
        in_maps.append(m)
    res = run_bass_kernel_spmd(nc, in_maps, list(range(N_CORES)),
                               trace=trace, **kwargs)
    out = np.concatenate([res.results[c]["out"] for c in range(N_CORES)], axis=0)
    return out, res


def kernel(**inputs) -> np.ndarray:
    out, _ = run(inputs)
    return out
